# revision 1
# baseline (speedup 1.0000x reference)
"""LBP (local binary pattern) extractor on 8 Trainium2 NeuronCores.

Reference semantics (for each pixel p and its 8 neighbors n_k in clockwise
order with weights 1,2,4,...,128):
    bit_k = (img[p + off_k] >= img[p]),  where index -1 wraps (python
    negative indexing) and index >= size contributes 0.
    out = sum_k w_k * bit_k   (uint8)

Strategy:
  * Shard rows across 8 cores (1024 rows each) - embarrassingly parallel.
  * Host builds a padded slab per core: +1 halo row top/bottom and +1 halo
    col left/right.  Low-edge halos carry the wrapped row/col (python -1
    indexing); high-edge halos carry a -3e38 sentinel so `neighbor >= center`
    is identically False (the reference's IndexError -> bit 0 case).  This
    makes the device kernel completely uniform - no edge special-casing.
  * Device kernel per tile ([128 out rows] x [CW cols]):
      - DMA three row-shifted fp32 copies (up/center/down) into SBUF, so
        every engine access pattern starts at partition 0 (HW constraint:
        engine SBUF APs may only start at partitions 0/32/64/96).
      - 8x DVE tensor_tensor(is_ge) with column-shifted access patterns
        -> 8 bf16 0/1 bitplanes.
      - PE merges the 8 planes with weighted-identity matmuls accumulating
        in PSUM (weights 2^k on the diagonals) - byte assembly is free.
      - ACT copies PSUM -> uint8 SBUF, DMA out.
"""

import numpy as np

H = 8192
W = 8192
NCORES = 8
RPC = H // NCORES  # rows per core

CW = 2048  # columns per tile
TR = 128  # output rows per row tile
MMW = 512  # matmul moving free dim

# (dx, dy, weight) in the reference's clockwise order
OFFSETS = [
    (-1, -1, 1), (-1, 0, 2), (-1, 1, 4), (0, 1, 8),
    (1, 1, 16), (1, 0, 32), (1, -1, 64), (0, -1, 128),
]

SENTINEL = -3.0e38  # < any finite image value


def _build_bass():
    import concourse.bacc as bacc
    import concourse.mybir as mybir
    from concourse.tile import TileContext

    f32 = mybir.dt.float32
    bf16 = mybir.dt.bfloat16
    u8 = mybir.dt.uint8

    nc = bacc.Bacc("TRN2", target_bir_lowering=False)
    x = nc.dram_tensor("x", [RPC + 2, W + 2], f32, kind="ExternalInput")
    wident = nc.dram_tensor("wident", [128, 8 * 128], bf16, kind="ExternalInput")
    y = nc.dram_tensor("y", [RPC, W], u8, kind="ExternalOutput")

    n_row_tiles = (RPC + TR - 1) // TR
    n_col_chunks = W // CW

    with TileContext(nc) as tc:
        with (
            tc.tile_pool(name="const", bufs=1) as cpool,
            tc.tile_pool(name="img", bufs=2) as ipool,
            tc.tile_pool(name="planes", bufs=2) as ppool,
            tc.tile_pool(name="outb", bufs=3) as opool,
            tc.tile_pool(name="psum", bufs=8, space="PSUM") as qpool,
        ):
            wid = cpool.tile([128, 8 * 128], bf16)
            nc.sync.dma_start(wid[:, :], wident[:, :])

            for rt in range(n_row_tiles):
                r0 = rt * TR
                tr = min(TR, RPC - r0)
                for ct in range(n_col_chunks):
                    c0 = ct * CW
                    # img_s[d][p, :] = padded slab row (r0 + p + d), i.e.
                    # image row (r0 + p + d - 1): d=0 up, d=1 center, d=2 down
                    img_s = []
                    for d in range(3):
                        t = ipool.tile([128, CW + 2], f32, tag=f"img{d}")
                        nc.sync.dma_start(
                            t[0:tr, :], x[r0 + d : r0 + d + tr, c0 : c0 + CW + 2]
                        )
                        img_s.append(t)
                    ctr = img_s[1]
                    planes = []
                    for dx, dy, _w in OFFSETS:
                        pl = ppool.tile([128, CW], bf16, tag=f"pl{dx}{dy}")
                        nc.vector.tensor_tensor(
                            out=pl[0:tr, :],
                            in0=img_s[1 + dx][0:tr, 1 + dy : 1 + dy + CW],
                            in1=ctr[0:tr, 1 : 1 + CW],
                            op=mybir.AluOpType.is_ge,
                        )
                        planes.append(pl)
                    ou = opool.tile([128, CW], u8, tag="out")
                    for q in range(CW // MMW):
                        ps = qpool.tile([128, MMW], f32, tag="ps")
                        for k in range(8):
                            nc.tensor.matmul(
                                ps[0:tr, :],
                                lhsT=wid[0:tr, 128 * k : 128 * k + tr],
                                rhs=planes[k][0:tr, q * MMW : (q + 1) * MMW],
                                start=(k == 0),
                                stop=(k == 7),
                            )
                        nc.scalar.copy(
                            ou[0:tr, q * MMW : (q + 1) * MMW], ps[0:tr, :]
                        )
                    nc.sync.dma_start(y[r0 : r0 + tr, c0 : c0 + CW], ou[0:tr, :])

    nc.compile()
    return nc


_NC_CACHE = None


def _get_nc():
    global _NC_CACHE
    if _NC_CACHE is None:
        _NC_CACHE = _build_bass()
    return _NC_CACHE


def _host_inputs(img: np.ndarray):
    import ml_dtypes

    pad = np.full((H + 2, W + 2), SENTINEL, np.float32)
    pad[1 : H + 1, 1 : W + 1] = img
    pad[0, 1 : W + 1] = img[H - 1]  # top wrap row
    pad[1 : H + 1, 0] = img[:, W - 1]  # left wrap col
    pad[0, 0] = img[H - 1, W - 1]  # NW corner wrap
    # bottom row / right col stay at the sentinel (invalid-high -> bit 0)

    widf = np.zeros((128, 8 * 128), np.float32)
    idx = np.arange(128)
    for k, (_dx, _dy, wgt) in enumerate(OFFSETS):
        widf[idx, 128 * k + idx] = float(wgt)
    wid = widf.astype(ml_dtypes.bfloat16)

    in_maps = []
    for c in range(NCORES):
        in_maps.append(
            {
                "x": np.ascontiguousarray(pad[RPC * c : RPC * c + RPC + 2, :]),
                "wident": wid,
            }
        )
    return in_maps


def kernel(rgb_image: np.ndarray, _trace: bool = False, _tmpdir: str | None = None):
    from concourse import bass_utils

    img = np.asarray(rgb_image, dtype=np.float32)
    assert img.shape == (H, W), img.shape
    in_maps = _host_inputs(img)
    nc = _get_nc()
    try:
        res = bass_utils.run_bass_kernel_spmd(
            nc,
            in_maps,
            core_ids=list(range(NCORES)),
            trace=_trace,
            tmpdir=_tmpdir,
        )
    except ModuleNotFoundError:
        # axon NTFF profile hook unavailable -> run without trace
        res = bass_utils.run_bass_kernel_spmd(
            nc, in_maps, core_ids=list(range(NCORES)), trace=False
        )
    out = np.concatenate([r["y"] for r in res.results], axis=0)
    if _trace:
        kernel.last_results = res
    return out

